# revision 9
# baseline (speedup 1.0000x reference)
"""CTGRUCell Trainium2 kernel — tensor-parallel over units across 8 NeuronCores.

Reference computation (B=1024, input=512, N=512 units, M=8 timescales):
    h     = state.reshape(B, N, M);  h_sum = h.sum(-1)
    fused = [inputs, h_sum]                    # (B, 1024)
    z     = sigmoid(fused @ W_z + b_z)         # (B, N, M)
    r     = sigmoid(fused @ W_r + b_r)
    r_h   = (r * h).sum(-1)
    h_hat = tanh([inputs, r_h] @ W_h + b_h)
    h_new = (1-z)*h + z*alpha*h_hat            # alpha = softmax(-ln_tau) over M
    out   = h_new.sum(-1);  new_state = h_new.reshape(B, N*M)

Distribution: each core owns 64 units = 512 contiguous columns of W_*/state.
Everything lives transposed on-device ([cols, B] layout) so weights load
untransposed, biases/alpha are per-partition, and the per-unit M-reductions
become tiny PE matmuls against a 0/1 selection matrix. Two 256 KiB
AllGathers (h_sum^T, r_h^T) stitch the cores together.
"""

import numpy as np

import concourse.bass as bass
import concourse.bacc as bacc
import concourse.mybir as mybir
from concourse.bass import ts
from concourse.tile import TileContext

# Problem shape (hardcoded per contract)
B = 1024
INPUT = 512
NUNITS = 512
MSCALES = 8
FAN = INPUT + NUNITS        # 1024
NM = NUNITS * MSCALES       # 4096
NCORES = 8
CU = NUNITS // NCORES       # 64 units per core
CCOLS = CU * MSCALES        # 512 state/W columns per core
P = 128
KO = FAN // P               # 8 contraction chunks
XC = INPUT // P             # 4 chunks of inputs^T
CC = CCOLS // P             # 4 column chunks per core
NHALF = 2                   # B split into two 512-wide matmul free dims

f32 = mybir.dt.float32
f32r = mybir.dt.float32r


def _build_nc():
    nc = bacc.Bacc(None, num_devices=NCORES)
    SIG = mybir.ActivationFunctionType.Sigmoid
    TANH = mybir.ActivationFunctionType.Tanh
    CPY = mybir.ActivationFunctionType.Copy
    MUL = mybir.AluOpType.mult
    ADD = mybir.AluOpType.add
    SUB = mybir.AluOpType.subtract

    # ---- I/O ----
    xT = nc.declare_dram_parameter("xT", [INPUT, B], f32r, isOutput=False)
    hT = nc.declare_dram_parameter("hT", [CCOLS, B], f32r, isOutput=False)
    wz = nc.declare_dram_parameter("wz", [FAN, CCOLS], f32r, isOutput=False)
    wr = nc.declare_dram_parameter("wr", [FAN, CCOLS], f32r, isOutput=False)
    wh = nc.declare_dram_parameter("wh", [FAN, CCOLS], f32r, isOutput=False)
    bz = nc.declare_dram_parameter("bz", [CCOLS], f32, isOutput=False)
    br = nc.declare_dram_parameter("br", [CCOLS], f32, isOutput=False)
    bh = nc.declare_dram_parameter("bh", [CCOLS], f32, isOutput=False)
    ssel = nc.declare_dram_parameter("ssel", [P, CC, CU], f32r, isOutput=False)
    acol = nc.declare_dram_parameter("acol", [P, 1], f32, isOutput=False)
    nsT = nc.declare_dram_parameter("nsT", [CCOLS, B], f32, isOutput=True)
    outT = nc.declare_dram_parameter("outT", [CU, B], f32, isOutput=True)

    with TileContext(nc) as tc:
        with (
            tc.tile_pool(name="sb", bufs=1) as sb,
            tc.tile_pool(name="psg", bufs=4, space="PSUM") as psg,
            tc.tile_pool(name="psr", bufs=2, space="PSUM") as psr,
            tc.tile_pool(name="dr", bufs=1, space="DRAM") as dr,
        ):
            # ---- persistent SBUF tensors ----
            hT_sb = sb.tile([P, CC, B], f32r)
            ssel_sb = sb.tile([P, CC, CU], f32r)
            acol_sb = sb.tile([P, 1], f32)
            bz_sb = sb.tile([P, CC], f32)
            br_sb = sb.tile([P, CC], f32)
            bh_sb = sb.tile([P, CC], f32)
            xT_sb = sb.tile([P, XC, B], f32r)
            wr_sb = sb.tile([P, KO, CCOLS], f32r)
            wz_sb = sb.tile([P, KO, CCOLS], f32r)
            wh_sb = sb.tile([P, KO, CCOLS], f32r)
            hsT_sb = sb.tile([P, XC, B], f32r)   # gathered h_sum^T (rows 512..1023 of fused)
            rhT_sb = sb.tile([P, XC, B], f32r)   # gathered r_h^T
            zT_sb = sb.tile([P, CC, B], f32)     # sigmoid(z); later alpha*z
            rT_sb = sb.tile([P, CC, B], f32r)    # sigmoid(r); later r*h
            hhT_sb = sb.tile([P, CC, B], f32)    # tanh(h_hat)
            qT_sb = sb.tile([P, CC, B], f32)     # (1-z)*h
            hsc_sb = sb.tile([CU, B], f32)       # local h_sum^T shard
            rhc_sb = sb.tile([CU, B], f32)       # local r_h^T shard
            outc_sb = sb.tile([CU, B], f32)      # local output shard

            # ---- loads (chunked so each matmul waits on few DMA queue sems) ----
            hT_r = hT.rearrange("(c p) b -> p c b", p=P)
            for c in range(CC):
                nc.sync.dma_start(hT_sb[:, c], hT_r[:, c])
            nc.sync.dma_start(ssel_sb[:], ssel[:])
            nc.gpsimd.dma_start(acol_sb[:], acol[:])
            nc.gpsimd.dma_start(bz_sb[:], bz.rearrange("(c p) -> p c", p=P))
            nc.gpsimd.dma_start(br_sb[:], br.rearrange("(c p) -> p c", p=P))
            nc.gpsimd.dma_start(bh_sb[:], bh.rearrange("(c p) -> p c", p=P))
            wr_r = wr.rearrange("(k p) c -> p k c", p=P)
            wz_r = wz.rearrange("(k p) c -> p k c", p=P)
            wh_r = wh.rearrange("(k p) c -> p k c", p=P)
            xT_r = xT.rearrange("(k p) b -> p k b", p=P)
            for k in range(KO):
                nc.sync.dma_start(wr_sb[:, k], wr_r[:, k])
            for k in range(XC):
                nc.sync.dma_start(xT_sb[:, k], xT_r[:, k])
            for k in range(KO):
                nc.sync.dma_start(wz_sb[:, k], wz_r[:, k])
            for k in range(KO):
                nc.sync.dma_start(wh_sb[:, k], wh_r[:, k])

            rg = [list(range(NCORES))]

            def msum_reduce(src_sb, dst_sb):
                """dst[u, b] = sum_m src[(u, m), b] via selection-matrix matmuls."""
                for h in range(NHALF):
                    ps = psr.tile([CU, 512], f32, name="ps_red")
                    for c in range(CC):
                        nc.tensor.matmul(
                            ps[:], ssel_sb[:, c], src_sb[:, c, ts(h, 512)],
                            start=(c == 0), stop=(c == CC - 1))
                    nc.scalar.activation(dst_sb[:, ts(h, 512)], ps[:], CPY)

            def allgather(local_sb, dest_sb, tag):
                ag_in = dr.tile([CU, B], f32, name=f"agi_{tag}")
                ag_out = dr.tile([NUNITS, B], f32, addr_space="Shared",
                                 name=f"ago_{tag}")
                nc.gpsimd.dma_start(ag_in[:], local_sb[:])
                nc.gpsimd.collective_compute(
                    "AllGather", mybir.AluOpType.bypass,
                    replica_groups=rg, ins=[ag_in[:]], outs=[ag_out[:]])
                ago_r = ag_out.rearrange("(k p) b -> p k b", p=P)
                for k in range(XC):
                    nc.gpsimd.dma_start(dest_sb[:, k], ago_r[:, k])

            def gate_matmuls(w_sb, b_sb, rhs_hi, act, dst_sb, dst_dtype_f32r=False):
                """dst[(cc,p), b] = act(sum_k w[k, (cc,p)] * fused[k, b] + bias)."""
                for cc in range(CC):
                    for h in range(NHALF):
                        ps = psg.tile([P, 512], f32, name="ps_gate")
                        for k in range(KO):
                            rhs = xT_sb[:, k] if k < XC else rhs_hi[:, k - XC]
                            nc.tensor.matmul(
                                ps[:], w_sb[:, k, ts(cc, P)], rhs[:, ts(h, 512)],
                                start=(k == 0), stop=(k == KO - 1))
                        nc.scalar.activation(
                            dst_sb[:, cc, ts(h, 512)], ps[:], act,
                            bias=b_sb[:, cc:cc + 1])

            # ---- phase A: h_sum shard + AllGather ----
            msum_reduce(hT_sb, hsc_sb)
            allgather(hsc_sb, hsT_sb, "hs")

            # ---- phase B: r gate ----
            gate_matmuls(wr_sb, br_sb, hsT_sb, SIG, rT_sb)

            # ---- phase C: r*h, reduce, AllGather ----
            for c in range(CC):
                nc.vector.tensor_tensor(
                    rT_sb[:, c], rT_sb[:, c].bitcast(f32), hT_sb[:, c].bitcast(f32),
                    MUL)
            msum_reduce(rT_sb, rhc_sb)
            allgather(rhc_sb, rhT_sb, "rh")

            # ---- phase D: z gate (overlaps AllGather #2) ----
            gate_matmuls(wz_sb, bz_sb, hsT_sb, SIG, zT_sb)

            # DVE prework that only needs z and h:
            #   q = (1-z)*h ; z <- alpha*z
            for c in range(CC):
                nc.vector.tensor_scalar(
                    qT_sb[:, c], zT_sb[:, c], -1.0, 1.0, MUL, ADD)
                nc.vector.tensor_tensor(
                    qT_sb[:, c], qT_sb[:, c], hT_sb[:, c].bitcast(f32), MUL)
                nc.vector.tensor_scalar_mul(zT_sb[:, c], zT_sb[:, c], acol_sb[:, 0:1])

            # ---- phase E: h_hat gate ----
            gate_matmuls(wh_sb, bh_sb, rhT_sb, TANH, hhT_sb)

            # ---- phase F: combine  h_new = q + (alpha*z)*h_hat ----
            nsT_r = nsT.rearrange("(c p) b -> p c b", p=P)
            for c in range(CC):
                nc.vector.tensor_tensor(
                    hhT_sb[:, c], zT_sb[:, c], hhT_sb[:, c], MUL)
                nc.vector.tensor_tensor(
                    hT_sb[:, c], qT_sb[:, c], hhT_sb[:, c], ADD)
                nc.sync.dma_start(nsT_r[:, c], hT_sb[:, c].bitcast(f32))

            # ---- phase G: output = sum_m h_new ----
            msum_reduce(hT_sb, outc_sb)
            nc.sync.dma_start(outT[:], outc_sb[:])

    nc.compile()
    return nc


_CACHE = {}


def _get_nc():
    if "nc" not in _CACHE:
        _CACHE["nc"] = _build_nc()
    return _CACHE["nc"]


def make_in_maps(inputs, state, W_z, b_z, W_r, b_r, W_h, b_h):
    """Host-side sharding: returns per-core input dicts."""
    inputs = np.ascontiguousarray(inputs, dtype=np.float32)
    state = np.ascontiguousarray(state, dtype=np.float32)
    xT = np.ascontiguousarray(inputs.T)                  # (512, B)
    stateT = np.ascontiguousarray(state.T)               # (4096, B)

    ssel = np.zeros((P, CC, CU), dtype=np.float32)
    for p in range(P):
        for c in range(CC):
            ssel[p, c, (c * P + p) // MSCALES] = 1.0
    alpha = np.exp(-0.5 * np.log(np.float32(10.0)) *
                   np.arange(MSCALES, dtype=np.float32))
    alpha = (alpha / alpha.sum()).astype(np.float32)
    acol = np.ascontiguousarray(
        np.tile(alpha, P // MSCALES).reshape(P, 1))

    in_maps = []
    for c in range(NCORES):
        sl = slice(c * CCOLS, (c + 1) * CCOLS)
        in_maps.append({
            "xT": xT,
            "hT": np.ascontiguousarray(stateT[sl]),
            "wz": np.ascontiguousarray(W_z[:, sl]),
            "wr": np.ascontiguousarray(W_r[:, sl]),
            "wh": np.ascontiguousarray(W_h[:, sl]),
            "bz": np.ascontiguousarray(b_z[sl], dtype=np.float32),
            "br": np.ascontiguousarray(b_r[sl], dtype=np.float32),
            "bh": np.ascontiguousarray(b_h[sl], dtype=np.float32),
            "ssel": ssel,
            "acol": acol,
        })
    return in_maps


def assemble(results):
    """Gather per-core outputs back to full (output, new_state)."""
    output = np.empty((B, NUNITS), dtype=np.float32)
    new_state = np.empty((B, NM), dtype=np.float32)
    for c in range(NCORES):
        output[:, c * CU:(c + 1) * CU] = results[c]["outT"].T
        new_state[:, c * CCOLS:(c + 1) * CCOLS] = results[c]["nsT"].T
    return output, new_state


def _get_runner():
    """Build (once) a jitted 8-core runner: in_maps-concat arrays -> out dict.

    Mirrors bass2jax.run_bass_via_pjrt's multi-core path, but caches the
    traced jit so repeated kernel() calls skip retracing, and exposes an
    n_steps chain (new_state fed back as state) for steady-state timing.
    """
    if "runner" in _CACHE:
        return _CACHE["runner"]

    import jax
    import jax.numpy as jnp
    from jax.experimental.shard_map import shard_map
    from jax.sharding import Mesh, PartitionSpec
    from concourse import bass2jax
    import concourse.mybir as _mybir

    nc = _get_nc()
    bass2jax.install_neuronx_cc_hook()

    partition_name = nc.partition_id_tensor.name if nc.partition_id_tensor else None
    in_names, out_names, out_avals = [], [], []
    for alloc in nc.m.functions[0].allocations:
        if not isinstance(alloc, _mybir.MemoryLocationSet):
            continue
        name = alloc.memorylocations[0].name
        if alloc.kind == "ExternalInput":
            if name != partition_name:
                in_names.append(name)
        elif alloc.kind == "ExternalOutput":
            out_names.append(name)
            out_avals.append(jax.core.ShapedArray(
                tuple(alloc.tensor_shape), _mybir.dt.np(alloc.dtype)))
    n_params = len(in_names)
    all_in_names = tuple(in_names + out_names +
                         ([partition_name] if partition_name else []))

    def _body(args, zeros):
        operands = list(args) + list(zeros)
        if partition_name is not None:
            operands.append(bass2jax.partition_id_tensor())
        outs = bass2jax._bass_exec_p.bind(
            *operands,
            out_avals=tuple(out_avals),
            in_names=all_in_names,
            out_names=tuple(out_names),
            lowering_input_output_aliases=(),
            sim_require_finite=True,
            sim_require_nnan=True,
            nc=nc,
        )
        return dict(zip(out_names, outs))

    hT_idx = in_names.index("hT")
    ns_name = "nsT"

    def _step(*all_args):
        args = list(all_args[:n_params])
        zeros = list(all_args[n_params:])
        return _body(args, zeros)

    devices = jax.devices()[:NCORES]
    mesh = Mesh(np.asarray(devices), ("core",))
    n_outs = len(out_names)

    jitted = jax.jit(shard_map(
        _step, mesh=mesh,
        in_specs=(PartitionSpec("core"),) * (n_params + n_outs),
        out_specs={n: PartitionSpec("core") for n in out_names},
        check_rep=False))

    def run(in_maps, n_steps=1, device_args=None):
        """Returns (per-core results, device_args) after n_steps chained steps."""
        if device_args is None:
            concat_in = [
                np.concatenate([np.asarray(in_maps[c][name])
                                for c in range(NCORES)], axis=0)
                for name in in_names
            ]
            concat_in += [
                np.zeros((NCORES * a.shape[0], *a.shape[1:]), a.dtype)
                for a in out_avals
            ]
            device_args = [jax.device_put(a) for a in concat_in]
        args = list(device_args)
        out = jitted(*args)
        for _ in range(n_steps - 1):
            args[hT_idx] = out[ns_name]
            out = jitted(*args)
        jax.block_until_ready(out)
        out = {k: np.asarray(v) for k, v in out.items()}
        return [
            {name: out[name].reshape(NCORES, *out_avals[i].shape)[c]
             for i, name in enumerate(out_names)}
            for c in range(NCORES)
        ], device_args

    _CACHE["runner"] = run
    return run


def kernel(inputs, state, W_z, b_z, W_r, b_r, W_h, b_h):
    run = _get_runner()
    in_maps = make_in_maps(inputs, state, W_z, b_z, W_r, b_r, W_h, b_h)
    results, _ = run(in_maps, n_steps=1)
    return assemble(results)
